# revision 15
# baseline (speedup 1.0000x reference)
"""Bilateral-grid slice kernel for Trainium2 (8 NeuronCores, SPMD data-parallel).

Strategy (per core):
  - shard: view v = core//4 owns grids[v]; quarter q = core%4 owns image rows
    [270q, 270(q+1)) of the 1080-row image -> 518400 pixels per core.
  - pixels live in "block layout" [128 partitions, 4050 free] (pixel = p*4050+j).
  - trilinear interp of the (8,16,16) grid:
      hat weights  hz[8], hy[16], hx[16]  with hat(t) = relu(1-|t|)
      S = hz (x) hy  joint one-hot over the 128 (z,y)-cells  (one DVE mul)
      S^T per 128-pixel tile via PE transpose (+ batched PSUM->SBUF copy
      that also rounds to fp32r)
      V[px, (ch,x)] = S @ G3 on the PE in fp32r  (G3 = grid [128, 192])
      x-interp: custom fused DVE op  P = prefix_scan(V * hx)  (one PSUM pass),
      chained across the 18 j's of a group; then per-group strided diffs
      A[n] = P[16n+15] - P[16n-1] recover the 12-channel affines.
      out = A[:, i*4+j] affine-applied to rgb (GPSIMD tensor ops)
"""

import dataclasses
import re

import numpy as np
from contextlib import ExitStack

import concourse.bacc as bacc
import concourse.bass as bass
import concourse.tile as tile
import concourse.mybir as mybir
from concourse import masks
from concourse import dve_ops as _dvo
from concourse.dve_spec import Spec, Src0, Src1, AluOp, C0
from concourse.dve_spec import scan as _dscan
from concourse.bass_utils import run_bass_kernel_spmd

F32 = mybir.dt.float32
F32R = mybir.dt.float32r
BF16 = mybir.dt.bfloat16
ALU = mybir.AluOpType
ACTFN = mybir.ActivationFunctionType

# problem geometry (hardcoded per contest rules)
NVIEW, L, GH, GW = 2, 8, 16, 16
IMG_H, IMG_W = 1080, 1920
NCORES = 8
P = 128

ROWS_PER_CORE = IMG_H // 4                      # 270
PIX_PER_CORE = ROWS_PER_CORE * IMG_W            # 518400
CTOT = PIX_PER_CORE // P                        # 4050
CHUNK = 162                                     # free-cols per chunk
NCHUNK = CTOT // CHUNK                          # 25
JGRP = 18                                       # j's per S-product group (even)
NGRP = CHUNK // JGRP                            # 9

GRAY_R, GRAY_G, GRAY_B = 0.299, 0.587, 0.114


def _register_scan_op():
    """Fused DVE op: out = prefix_scan_add(in0*in1) + s0  (s0 = chain seed)."""
    for existing in _dvo.OPS:
        if existing.name == "TT_MUL_PSCAN_ANT":
            return existing

    def _ref(in0, in1, s0, s1, imm2):
        p = in0.astype(np.float32) * in1.astype(np.float32)
        flat = p.reshape(p.shape[0], -1)
        out = np.cumsum(flat, axis=1) + s0
        return out.reshape(in0.shape).astype(np.float32)

    spec = Spec(body=_dscan(AluOp.ADD, Src0 * Src1, init=C0), reference=_ref)
    op = _dvo.DveOp(
        "TT_MUL_PSCAN_ANT", spec, subdim=False,
        uops_sha={"v3": "738a75e9e385e48e", "v4": "f4b949e6ae385ae2"},
    )
    _dvo.OPS.append(op)
    _dvo._SUB_OPCODE_FOR_NAME[op.name] = (
        _dvo._CUSTOM_DVE_ROW_BASE + len(_dvo.OPS) - 1
    )
    _dvo.CUSTOM_DVE_SPECS[op.name] = spec
    # re-pin shas if the in-repo lowering drifted from the hardcoded ones
    shas = {}
    for ver in ("v3", "v4"):
        try:
            op.compile(ver)
            shas[ver] = op.uops_sha[ver]
        except ValueError as e:
            m = re.search(r"\((v\d+): ([0-9a-f]+) ", str(e))
            if m:
                shas[ver] = m.group(2)
    if shas != op.uops_sha:
        op = dataclasses.replace(op, uops_sha=shas)
        _dvo.OPS[-1] = op
        _dvo.CUSTOM_DVE_SPECS[op.name] = op.spec
    return op


SCAN_OP = _register_scan_op()


def _ap(base: bass.AP, offset_add: int, free_dims):
    """Raw AP on the same tensor/partitions as `base` with custom free dims."""
    return bass.AP(base.tensor, base.offset + offset_add, [base.ap[0]] + free_dims)


def build_module(ctot=CTOT, chunk=CHUNK, jgrp=JGRP, use_f32r=True):
    nchunk = ctot // chunk
    ngrp = chunk // jgrp
    assert ctot % chunk == 0 and chunk % jgrp == 0 and jgrp % 2 == 0

    assert jgrp % 4 == 0 or jgrp % 2 == 0
    nc = bacc.Bacc("TRN2", target_bir_lowering=False, debug=False,
                   num_devices=NCORES)

    xs = nc.dram_tensor("xs", [P, ctot], F32, kind="ExternalInput").ap()
    ys = nc.dram_tensor("ys", [P, ctot], F32, kind="ExternalInput").ap()
    rr = nc.dram_tensor("rr", [P, ctot], F32, kind="ExternalInput").ap()
    gg = nc.dram_tensor("gg", [P, ctot], F32, kind="ExternalInput").ap()
    bb = nc.dram_tensor("bb", [P, ctot], F32, kind="ExternalInput").ap()
    g3d = nc.dram_tensor("g3", [P, 192], F32, kind="ExternalInput").ap()
    cst = nc.dram_tensor("cst", [1, 40], F32, kind="ExternalInput").ap()
    out = nc.dram_tensor("out", [P, 3 * ctot], F32, kind="ExternalOutput").ap()

    MMDT = BF16

    with tile.TileContext(nc) as tc:
        with ExitStack() as ctx:
            cpool = ctx.enter_context(tc.tile_pool(name="const", bufs=1))
            inp = ctx.enter_context(tc.tile_pool(name="inp", bufs=2))
            hatp = ctx.enter_context(tc.tile_pool(name="hat", bufs=2))
            spool = ctx.enter_context(tc.tile_pool(name="sprod", bufs=2))
            stp = ctx.enter_context(
                tc.tile_pool(name="st_ps", bufs=2, space="PSUM"))
            stsb = ctx.enter_context(tc.tile_pool(name="st_sb", bufs=3))
            vps = ctx.enter_context(
                tc.tile_pool(name="v_ps", bufs=3, space="PSUM"))
            w2p = ctx.enter_context(tc.tile_pool(name="w2", bufs=2))
            apool = ctx.enter_context(tc.tile_pool(name="acc", bufs=2))
            opool = ctx.enter_context(tc.tile_pool(name="outb", bufs=2))

            # constants
            g3_f = cpool.tile([P, 192], F32)
            nc.sync.dma_start(g3_f[:], g3d)
            g3_sb = cpool.tile([P, 192], MMDT)
            nc.scalar.copy(g3_sb[:], g3_f[:])
            zc_sb = cpool.tile([P, 8], F32)
            nc.sync.dma_start(zc_sb[:], cst[0:1, 0:8].to_broadcast((P, 8)))
            yc_sb = cpool.tile([P, 16], F32)
            nc.sync.dma_start(yc_sb[:], cst[0:1, 8:24].to_broadcast((P, 16)))
            xc_sb = cpool.tile([P, 16], F32)
            nc.sync.dma_start(xc_sb[:], cst[0:1, 24:40].to_broadcast((P, 16)))
            ident_f = cpool.tile([P, P], F32)
            masks.make_identity(nc, ident_f[:])

            for ci in range(nchunk):
                cb = ci * chunk
                xt = inp.tile([P, chunk], F32, tag="xt")
                nc.sync.dma_start(xt[:], xs[:, cb:cb + chunk])
                yt = inp.tile([P, chunk], F32, tag="yt")
                nc.sync.dma_start(yt[:], ys[:, cb:cb + chunk])
                rt = inp.tile([P, chunk], F32, tag="rt")
                nc.sync.dma_start(rt[:], rr[:, cb:cb + chunk])
                gt = inp.tile([P, chunk], F32, tag="gt")
                nc.sync.dma_start(gt[:], gg[:, cb:cb + chunk])
                bt = inp.tile([P, chunk], F32, tag="bt")
                nc.sync.dma_start(bt[:], bb[:, cb:cb + chunk])

                # gray precursor (z = t2 * 0.587*7 folded into the hat STT)
                t1 = inp.tile([P, chunk], F32, tag="t1")
                nc.vector.scalar_tensor_tensor(
                    t1[:], rt[:], GRAY_R / GRAY_G, gt[:],
                    op0=ALU.mult, op1=ALU.add)
                t2 = inp.tile([P, chunk], F32, tag="t2")
                nc.vector.scalar_tensor_tensor(
                    t2[:], bt[:], GRAY_B / GRAY_G, t1[:],
                    op0=ALU.mult, op1=ALU.add)

                # hat argument tiles, free layout (j, cell) j-major;
                # the arg STTs run on GPSIMD to relieve the Vector engine
                hz = hatp.tile([P, 8 * chunk], F32, tag="hz")
                nc.vector.scalar_tensor_tensor(
                    hz[:].rearrange("p (j z) -> p j z", z=8),
                    t2[:].unsqueeze(2).broadcast_to((P, chunk, 8)),
                    GRAY_G * (L - 1),
                    zc_sb[:].unsqueeze(1).broadcast_to((P, chunk, 8)),
                    op0=ALU.mult, op1=ALU.subtract)
                hy = hatp.tile([P, 16 * chunk], F32, tag="hy")
                nc.vector.scalar_tensor_tensor(
                    hy[:].rearrange("p (j y) -> p j y", y=16),
                    yt[:].unsqueeze(2).broadcast_to((P, chunk, 16)),
                    float(GH - 1),
                    yc_sb[:].unsqueeze(1).broadcast_to((P, chunk, 16)),
                    op0=ALU.mult, op1=ALU.subtract)
                hx = hatp.tile([P, 16 * chunk], F32, tag="hx")
                nc.vector.scalar_tensor_tensor(
                    hx[:].rearrange("p (j x) -> p j x", x=16),
                    xt[:].unsqueeze(2).broadcast_to((P, chunk, 16)),
                    float(GW - 1),
                    xc_sb[:].unsqueeze(1).broadcast_to((P, chunk, 16)),
                    op0=ALU.mult, op1=ALU.subtract)

                # hat(t) = relu(1 - |t|) on ACT; hz/hy land in bf16 for the
                # S-product / PE path, hx stays fp32 (feeds the scan)
                hzb = hatp.tile([P, 8 * chunk], BF16, tag="hzb")
                hyb = hatp.tile([P, 16 * chunk], BF16, tag="hyb")
                for h, hb in ((hz, hzb), (hy, hyb), (hx, None)):
                    nc.scalar.activation(h[:], h[:], ACTFN.Abs)
                    dst = h if hb is None else hb
                    nc.scalar.activation(dst[:], h[:], ACTFN.Relu,
                                         bias=1.0, scale=-1.0)

                a_ch = apool.tile([P, 12 * chunk], F32, tag="a_ch")

                for g in range(ngrp):
                    jb = g * jgrp
                    sg = spool.tile([P, jgrp * P], F32, tag="sg")
                    nc.vector.tensor_tensor(
                        sg[:].rearrange("p (j z y) -> p j z y", j=jgrp, z=8),
                        _ap(hzb[:], jb * 8, [[8, jgrp], [1, 8], [0, 16]]),
                        _ap(hyb[:], jb * 16, [[16, jgrp], [0, 8], [1, 16]]),
                        op=ALU.mult)

                    # per-group prefix tile: 18 chained scans of 192 cols
                    w2g = w2p.tile([P, jgrp * 192], F32, tag="w2g")

                    kb = 0
                    for bs in (4, 4, 4, 4, 2):
                        # `bs` transposes fill one PSUM bank; one batched
                        # rounding copy to bf16 SBUF
                        st_ps = stp.tile([P, 512], F32)
                        for q in range(bs):
                            jj = kb + q
                            nc.tensor.transpose(
                                _ap(st_ps[:], q * P, [[1, P]]),
                                sg[:, jj * P:(jj + 1) * P], ident_f[:])
                        st_sb = stsb.tile([P, 512], MMDT)
                        nc.scalar.copy(st_sb[:, :bs * P], st_ps[:, :bs * P])

                        for h in range(bs // 2):
                            vt = vps.tile([P, 1024], F32)
                            for q in range(2):
                                nc.tensor.matmul(
                                    _ap(vt[:], q * 512, [[1, 192]]),
                                    lhsT=_ap(st_sb[:], (h * 2 + q) * P,
                                             [[1, P]]),
                                    rhs=g3_sb[:], start=True, stop=True)

                            # fused x-interp: chained prefix scan of V * hx
                            for q in range(2):
                                k = kb + h * 2 + q
                                seed = (0.0 if k == 0
                                        else _ap(w2g[:], k * 192 - 1,
                                                 [[1, 1]]))
                                nc.vector._custom_dve(
                                    SCAN_OP,
                                    out=_ap(w2g[:], k * 192, [[1, 192]]),
                                    in0=_ap(vt[:], q * 512, [[1, 192]]),
                                    in1=_ap(hx[:], (jb + k) * 16,
                                            [[0, 12], [1, 16]]),
                                    s0=seed)
                        kb += bs

                    # recover A[n] = P[16n+15] - P[16n-1] for the group's
                    # 216 (j, ch) pairs; n = 0 seeds from zero.
                    nc.vector.tensor_tensor(
                        _ap(a_ch[:], jb * 12 + 1, [[1, 12 * jgrp - 1]]),
                        _ap(w2g[:], 31, [[16, 12 * jgrp - 1]]),
                        _ap(w2g[:], 15, [[16, 12 * jgrp - 1]]),
                        op=ALU.subtract)
                    nc.vector.tensor_copy(
                        _ap(a_ch[:], jb * 12, [[1, 1]]),
                        _ap(w2g[:], 15, [[1, 1]]))

                # affine apply on GPSIMD:
                # out_i = A[4i]*r + A[4i+1]*g + A[4i+2]*b + A[4i+3]
                ot = opool.tile([P, 3 * chunk], F32, tag="ot")
                rgbt = (rt, gt, bt)
                for i in range(3):
                    m = []
                    for j in range(3):
                        mj = opool.tile([P, chunk], F32, tag=f"m{j}")
                        nc.gpsimd.tensor_tensor(
                            mj[:],
                            _ap(a_ch[:], 4 * i + j, [[12, chunk]]),
                            rgbt[j][:], op=ALU.mult)
                        m.append(mj)
                    s1 = opool.tile([P, chunk], F32, tag="s1")
                    nc.gpsimd.tensor_tensor(s1[:], m[0][:], m[1][:], op=ALU.add)
                    s2 = opool.tile([P, chunk], F32, tag="s2")
                    nc.gpsimd.tensor_tensor(
                        s2[:], m[2][:],
                        _ap(a_ch[:], 4 * i + 3, [[12, chunk]]), op=ALU.add)
                    nc.gpsimd.tensor_tensor(
                        _ap(ot[:], i, [[3, chunk]]), s1[:], s2[:], op=ALU.add)

                nc.sync.dma_start(out[:, 3 * cb:3 * (cb + chunk)], ot[:])

    nc.compile()
    return nc


_NC_CACHE = {}


def _get_module():
    key = (CTOT, CHUNK, JGRP)
    if key not in _NC_CACHE:
        _NC_CACHE[key] = build_module()
    return _NC_CACHE[key]


def _make_core_inputs(grids, coords, rgb, ctot=CTOT):
    """Per-core input dicts (numpy layout prep only)."""
    consts = np.concatenate([
        np.arange(8, dtype=np.float32),
        np.arange(16, dtype=np.float32),
        np.arange(16, dtype=np.float32),
    ]).reshape(1, 40)
    in_maps = []
    for core in range(NCORES):
        v, q = divmod(core, 4)
        r0, r1 = ROWS_PER_CORE * q, ROWS_PER_CORE * (q + 1)
        blk = lambda a: np.ascontiguousarray(a.reshape(P, ctot), np.float32)
        # G3[(zc*16+yc), (ch*16 + xc)] = grids[v, ch, zc, yc, xc]
        g3 = np.ascontiguousarray(
            grids[v].transpose(1, 2, 0, 3).reshape(P, 192), np.float32)
        in_maps.append({
            "xs": blk(coords[v, 0, r0:r1, :, 0]),
            "ys": blk(coords[v, 0, r0:r1, :, 1]),
            "rr": blk(rgb[v, 0, r0:r1, :, 0]),
            "gg": blk(rgb[v, 0, r0:r1, :, 1]),
            "bb": blk(rgb[v, 0, r0:r1, :, 2]),
            "g3": g3,
            "cst": consts,
        })
    return in_maps


def _run(grids, coords, rgb, trace=False):
    nc = _get_module()
    in_maps = _make_core_inputs(grids, coords, rgb)
    res = run_bass_kernel_spmd(nc, in_maps, core_ids=list(range(NCORES)),
                               trace=trace)
    outs = []
    for core in range(NCORES):
        o = res.results[core]["out"]
        outs.append(o.reshape(P, CTOT, 3).reshape(ROWS_PER_CORE, IMG_W, 3))
    full = np.empty((NVIEW, 1, IMG_H, IMG_W, 3), np.float32)
    for core in range(NCORES):
        v, q = divmod(core, 4)
        full[v, 0, ROWS_PER_CORE * q:ROWS_PER_CORE * (q + 1)] = outs[core]
    return full, res


def kernel(grids, coords, rgb):
    full, _ = _run(np.asarray(grids), np.asarray(coords), np.asarray(rgb))
    return full


# revision 17
# speedup vs baseline: 1.0483x; 1.0483x over previous
"""Bilateral-grid slice kernel for Trainium2 (8 NeuronCores, SPMD data-parallel).

Strategy (per core):
  - shard: view v = core//4 owns grids[v]; quarter q = core%4 owns image rows
    [270q, 270(q+1)) of the 1080-row image -> 518400 pixels per core.
  - pixels live in "block layout" [128 partitions, 4050 free] (pixel = p*4050+j).
  - trilinear interp of the (8,16,16) grid:
      hat weights  hz[8], hy[16], hx[16]  with hat(t) = relu(1-|t|)
      S = hz (x) hy  joint one-hot over the 128 (z,y)-cells  (one DVE mul)
      S^T per 128-pixel tile via PE transpose (+ batched PSUM->SBUF copy
      that also rounds to fp32r)
      V[px, (ch,x)] = S @ G3 on the PE in fp32r  (G3 = grid [128, 192])
      x-interp: custom fused DVE op  P = prefix_scan(V * hx)  (one PSUM pass),
      chained across the 18 j's of a group; then per-group strided diffs
      A[n] = P[16n+15] - P[16n-1] recover the 12-channel affines.
      out = A[:, i*4+j] affine-applied to rgb (GPSIMD tensor ops)
"""

import dataclasses
import re

import numpy as np
from contextlib import ExitStack

import concourse.bacc as bacc
import concourse.bass as bass
import concourse.tile as tile
import concourse.mybir as mybir
from concourse import masks
from concourse import dve_ops as _dvo
from concourse.dve_spec import Spec, Src0, Src1, AluOp, C0
from concourse.dve_spec import scan as _dscan
from concourse.bass_utils import run_bass_kernel_spmd

F32 = mybir.dt.float32
F32R = mybir.dt.float32r
BF16 = mybir.dt.bfloat16
ALU = mybir.AluOpType
ACTFN = mybir.ActivationFunctionType

# problem geometry (hardcoded per contest rules)
NVIEW, L, GH, GW = 2, 8, 16, 16
IMG_H, IMG_W = 1080, 1920
NCORES = 8
P = 128

ROWS_PER_CORE = IMG_H // 4                      # 270
PIX_PER_CORE = ROWS_PER_CORE * IMG_W            # 518400
CTOT = PIX_PER_CORE // P                        # 4050
CHUNK = 162                                     # free-cols per chunk
NCHUNK = CTOT // CHUNK                          # 25
JGRP = 18                                       # j's per S-product group (even)
NGRP = CHUNK // JGRP                            # 9

GRAY_R, GRAY_G, GRAY_B = 0.299, 0.587, 0.114


def _register_scan_op():
    """Fused DVE op: out = prefix_scan_add(in0*in1) + s0  (s0 = chain seed)."""
    for existing in _dvo.OPS:
        if existing.name == "TT_MUL_PSCAN_ANT":
            return existing

    def _ref(in0, in1, s0, s1, imm2):
        p = in0.astype(np.float32) * in1.astype(np.float32)
        flat = p.reshape(p.shape[0], -1)
        out = np.cumsum(flat, axis=1) + s0
        return out.reshape(in0.shape).astype(np.float32)

    spec = Spec(body=_dscan(AluOp.ADD, Src0 * Src1, init=C0), reference=_ref)
    op = _dvo.DveOp(
        "TT_MUL_PSCAN_ANT", spec, subdim=False,
        uops_sha={"v3": "738a75e9e385e48e", "v4": "f4b949e6ae385ae2"},
    )
    _dvo.OPS.append(op)
    _dvo._SUB_OPCODE_FOR_NAME[op.name] = (
        _dvo._CUSTOM_DVE_ROW_BASE + len(_dvo.OPS) - 1
    )
    _dvo.CUSTOM_DVE_SPECS[op.name] = spec
    # re-pin shas if the in-repo lowering drifted from the hardcoded ones
    shas = {}
    for ver in ("v3", "v4"):
        try:
            op.compile(ver)
            shas[ver] = op.uops_sha[ver]
        except ValueError as e:
            m = re.search(r"\((v\d+): ([0-9a-f]+) ", str(e))
            if m:
                shas[ver] = m.group(2)
    if shas != op.uops_sha:
        op = dataclasses.replace(op, uops_sha=shas)
        _dvo.OPS[-1] = op
        _dvo.CUSTOM_DVE_SPECS[op.name] = op.spec
    return op


SCAN_OP = _register_scan_op()


def _ap(base: bass.AP, offset_add: int, free_dims):
    """Raw AP on the same tensor/partitions as `base` with custom free dims."""
    return bass.AP(base.tensor, base.offset + offset_add, [base.ap[0]] + free_dims)


def build_module(ctot=CTOT, chunk=CHUNK, jgrp=JGRP, use_f32r=True):
    nchunk = ctot // chunk
    ngrp = chunk // jgrp
    assert ctot % chunk == 0 and chunk % jgrp == 0 and jgrp % 2 == 0

    assert jgrp % 4 == 0 or jgrp % 2 == 0
    nc = bacc.Bacc("TRN2", target_bir_lowering=False, debug=False,
                   num_devices=NCORES)

    xs = nc.dram_tensor("xs", [P, ctot], F32, kind="ExternalInput").ap()
    ys = nc.dram_tensor("ys", [P, ctot], F32, kind="ExternalInput").ap()
    rr = nc.dram_tensor("rr", [P, ctot], F32, kind="ExternalInput").ap()
    gg = nc.dram_tensor("gg", [P, ctot], F32, kind="ExternalInput").ap()
    bb = nc.dram_tensor("bb", [P, ctot], F32, kind="ExternalInput").ap()
    g3d = nc.dram_tensor("g3", [P, 192], F32, kind="ExternalInput").ap()
    cst = nc.dram_tensor("cst", [1, 40], F32, kind="ExternalInput").ap()
    out = nc.dram_tensor("out", [P, 3 * ctot], F32, kind="ExternalOutput").ap()

    MMDT = BF16

    with tile.TileContext(nc) as tc:
        with ExitStack() as ctx:
            cpool = ctx.enter_context(tc.tile_pool(name="const", bufs=1))
            inp = ctx.enter_context(tc.tile_pool(name="inp", bufs=2))
            hatp = ctx.enter_context(tc.tile_pool(name="hat", bufs=2))
            spool = ctx.enter_context(tc.tile_pool(name="sprod", bufs=1))
            stp = ctx.enter_context(
                tc.tile_pool(name="st_ps", bufs=2, space="PSUM"))
            stsb = ctx.enter_context(tc.tile_pool(name="st_sb", bufs=3))
            vps = ctx.enter_context(
                tc.tile_pool(name="v_ps", bufs=3, space="PSUM"))
            w2p = ctx.enter_context(tc.tile_pool(name="w2", bufs=2))
            apool = ctx.enter_context(tc.tile_pool(name="acc", bufs=2))
            opool = ctx.enter_context(tc.tile_pool(name="outb", bufs=2))

            # constants
            g3_f = cpool.tile([P, 192], F32)
            nc.sync.dma_start(g3_f[:], g3d)
            g3_sb = cpool.tile([P, 192], MMDT)
            nc.scalar.copy(g3_sb[:], g3_f[:])
            zc_sb = cpool.tile([P, 8], F32)
            nc.sync.dma_start(zc_sb[:], cst[0:1, 0:8].to_broadcast((P, 8)))
            yc_sb = cpool.tile([P, 16], F32)
            nc.sync.dma_start(yc_sb[:], cst[0:1, 8:24].to_broadcast((P, 16)))
            xc_sb = cpool.tile([P, 16], F32)
            nc.sync.dma_start(xc_sb[:], cst[0:1, 24:40].to_broadcast((P, 16)))
            ident_f = cpool.tile([P, P], F32)
            masks.make_identity(nc, ident_f[:])
            ident = cpool.tile([P, P], BF16)
            nc.vector.tensor_copy(ident[:], ident_f[:])

            for ci in range(nchunk):
                cb = ci * chunk
                xt = inp.tile([P, chunk], F32, tag="xt")
                nc.sync.dma_start(xt[:], xs[:, cb:cb + chunk])
                yt = inp.tile([P, chunk], F32, tag="yt")
                nc.sync.dma_start(yt[:], ys[:, cb:cb + chunk])
                rt = inp.tile([P, chunk], F32, tag="rt")
                nc.sync.dma_start(rt[:], rr[:, cb:cb + chunk])
                gt = inp.tile([P, chunk], F32, tag="gt")
                nc.sync.dma_start(gt[:], gg[:, cb:cb + chunk])
                bt = inp.tile([P, chunk], F32, tag="bt")
                nc.sync.dma_start(bt[:], bb[:, cb:cb + chunk])

                # gray precursor (z = t2 * 0.587*7 folded into the hat STT)
                t1 = inp.tile([P, chunk], F32, tag="t1")
                nc.vector.scalar_tensor_tensor(
                    t1[:], rt[:], GRAY_R / GRAY_G, gt[:],
                    op0=ALU.mult, op1=ALU.add)
                t2 = inp.tile([P, chunk], F32, tag="t2")
                nc.vector.scalar_tensor_tensor(
                    t2[:], bt[:], GRAY_B / GRAY_G, t1[:],
                    op0=ALU.mult, op1=ALU.add)

                # hat argument tiles, free layout (j, cell) j-major;
                # the arg STTs run on GPSIMD to relieve the Vector engine
                hz = hatp.tile([P, 8 * chunk], F32, tag="hz")
                nc.vector.scalar_tensor_tensor(
                    hz[:].rearrange("p (j z) -> p j z", z=8),
                    t2[:].unsqueeze(2).broadcast_to((P, chunk, 8)),
                    GRAY_G * (L - 1),
                    zc_sb[:].unsqueeze(1).broadcast_to((P, chunk, 8)),
                    op0=ALU.mult, op1=ALU.subtract)
                hy = hatp.tile([P, 16 * chunk], F32, tag="hy")
                nc.vector.scalar_tensor_tensor(
                    hy[:].rearrange("p (j y) -> p j y", y=16),
                    yt[:].unsqueeze(2).broadcast_to((P, chunk, 16)),
                    float(GH - 1),
                    yc_sb[:].unsqueeze(1).broadcast_to((P, chunk, 16)),
                    op0=ALU.mult, op1=ALU.subtract)
                hx = hatp.tile([P, 16 * chunk], F32, tag="hx")
                nc.vector.scalar_tensor_tensor(
                    hx[:].rearrange("p (j x) -> p j x", x=16),
                    xt[:].unsqueeze(2).broadcast_to((P, chunk, 16)),
                    float(GW - 1),
                    xc_sb[:].unsqueeze(1).broadcast_to((P, chunk, 16)),
                    op0=ALU.mult, op1=ALU.subtract)

                # hat(t) = relu(1 - |t|) on ACT; hz/hy land in bf16 for the
                # S-product / PE path, hx stays fp32 (feeds the scan)
                hzb = hatp.tile([P, 8 * chunk], BF16, tag="hzb")
                hyb = hatp.tile([P, 16 * chunk], BF16, tag="hyb")
                for h, hb in ((hz, hzb), (hy, hyb), (hx, None)):
                    nc.scalar.activation(h[:], h[:], ACTFN.Abs)
                    dst = h if hb is None else hb
                    nc.scalar.activation(dst[:], h[:], ACTFN.Relu,
                                         bias=1.0, scale=-1.0)

                a_ch = apool.tile([P, 12 * chunk], F32, tag="a_ch")

                # all 9 S-products up front so the Vector queue never
                # starves waiting on the PE/ACT round-trip at group starts
                sgs = []
                for g in range(ngrp):
                    jb = g * jgrp
                    sg = spool.tile([P, jgrp * P], BF16, tag=f"sg{g}")
                    nc.vector.tensor_tensor(
                        sg[:].rearrange("p (j z y) -> p j z y", j=jgrp, z=8),
                        _ap(hzb[:], jb * 8, [[8, jgrp], [1, 8], [0, 16]]),
                        _ap(hyb[:], jb * 16, [[16, jgrp], [0, 8], [1, 16]]),
                        op=ALU.mult)
                    sgs.append(sg)

                for g in range(ngrp):
                    jb = g * jgrp
                    sg = sgs[g]

                    # per-group prefix tile: 18 chained scans of 192 cols
                    w2g = w2p.tile([P, jgrp * 192], F32, tag="w2g")

                    kb = 0
                    for bs in (4, 4, 4, 4, 2):
                        # `bs` transposes fill one PSUM bank; one batched
                        # rounding copy to bf16 SBUF
                        st_ps = stp.tile([P, 512], BF16)
                        for q in range(bs):
                            jj = kb + q
                            nc.tensor.transpose(
                                _ap(st_ps[:], q * P, [[1, P]]),
                                sg[:, jj * P:(jj + 1) * P], ident[:])
                        st_sb = stsb.tile([P, 512], MMDT)
                        nc.scalar.copy(st_sb[:, :bs * P], st_ps[:, :bs * P])

                        for h in range(bs // 2):
                            vt = vps.tile([P, 1024], F32)
                            for q in range(2):
                                nc.tensor.matmul(
                                    _ap(vt[:], q * 512, [[1, 192]]),
                                    lhsT=_ap(st_sb[:], (h * 2 + q) * P,
                                             [[1, P]]),
                                    rhs=g3_sb[:], start=True, stop=True)

                            # fused x-interp: chained prefix scan of V * hx
                            for q in range(2):
                                k = kb + h * 2 + q
                                seed = (0.0 if k == 0
                                        else _ap(w2g[:], k * 192 - 1,
                                                 [[1, 1]]))
                                nc.vector._custom_dve(
                                    SCAN_OP,
                                    out=_ap(w2g[:], k * 192, [[1, 192]]),
                                    in0=_ap(vt[:], q * 512, [[1, 192]]),
                                    in1=_ap(hx[:], (jb + k) * 16,
                                            [[0, 12], [1, 16]]),
                                    s0=seed)
                        kb += bs

                    # recover A[n] = P[16n+15] - P[16n-1] for the group's
                    # 216 (j, ch) pairs; n = 0 seeds from zero.
                    nc.vector.tensor_tensor(
                        _ap(a_ch[:], jb * 12 + 1, [[1, 12 * jgrp - 1]]),
                        _ap(w2g[:], 31, [[16, 12 * jgrp - 1]]),
                        _ap(w2g[:], 15, [[16, 12 * jgrp - 1]]),
                        op=ALU.subtract)
                    nc.vector.tensor_copy(
                        _ap(a_ch[:], jb * 12, [[1, 1]]),
                        _ap(w2g[:], 15, [[1, 1]]))

                # affine apply on GPSIMD:
                # out_i = A[4i]*r + A[4i+1]*g + A[4i+2]*b + A[4i+3]
                ot = opool.tile([P, 3 * chunk], F32, tag="ot")
                rgbt = (rt, gt, bt)
                for i in range(3):
                    m = []
                    for j in range(3):
                        mj = opool.tile([P, chunk], F32, tag=f"m{j}")
                        nc.gpsimd.tensor_tensor(
                            mj[:],
                            _ap(a_ch[:], 4 * i + j, [[12, chunk]]),
                            rgbt[j][:], op=ALU.mult)
                        m.append(mj)
                    s1 = opool.tile([P, chunk], F32, tag="s1")
                    nc.gpsimd.tensor_tensor(s1[:], m[0][:], m[1][:], op=ALU.add)
                    s2 = opool.tile([P, chunk], F32, tag="s2")
                    nc.gpsimd.tensor_tensor(
                        s2[:], m[2][:],
                        _ap(a_ch[:], 4 * i + 3, [[12, chunk]]), op=ALU.add)
                    nc.gpsimd.tensor_tensor(
                        _ap(ot[:], i, [[3, chunk]]), s1[:], s2[:], op=ALU.add)

                nc.sync.dma_start(out[:, 3 * cb:3 * (cb + chunk)], ot[:])

    nc.compile()
    return nc


_NC_CACHE = {}


def _get_module():
    key = (CTOT, CHUNK, JGRP)
    if key not in _NC_CACHE:
        _NC_CACHE[key] = build_module()
    return _NC_CACHE[key]


def _make_core_inputs(grids, coords, rgb, ctot=CTOT):
    """Per-core input dicts (numpy layout prep only)."""
    consts = np.concatenate([
        np.arange(8, dtype=np.float32),
        np.arange(16, dtype=np.float32),
        np.arange(16, dtype=np.float32),
    ]).reshape(1, 40)
    in_maps = []
    for core in range(NCORES):
        v, q = divmod(core, 4)
        r0, r1 = ROWS_PER_CORE * q, ROWS_PER_CORE * (q + 1)
        blk = lambda a: np.ascontiguousarray(a.reshape(P, ctot), np.float32)
        # G3[(zc*16+yc), (ch*16 + xc)] = grids[v, ch, zc, yc, xc]
        g3 = np.ascontiguousarray(
            grids[v].transpose(1, 2, 0, 3).reshape(P, 192), np.float32)
        in_maps.append({
            "xs": blk(coords[v, 0, r0:r1, :, 0]),
            "ys": blk(coords[v, 0, r0:r1, :, 1]),
            "rr": blk(rgb[v, 0, r0:r1, :, 0]),
            "gg": blk(rgb[v, 0, r0:r1, :, 1]),
            "bb": blk(rgb[v, 0, r0:r1, :, 2]),
            "g3": g3,
            "cst": consts,
        })
    return in_maps


def _run(grids, coords, rgb, trace=False):
    nc = _get_module()
    in_maps = _make_core_inputs(grids, coords, rgb)
    res = run_bass_kernel_spmd(nc, in_maps, core_ids=list(range(NCORES)),
                               trace=trace)
    outs = []
    for core in range(NCORES):
        o = res.results[core]["out"]
        outs.append(o.reshape(P, CTOT, 3).reshape(ROWS_PER_CORE, IMG_W, 3))
    full = np.empty((NVIEW, 1, IMG_H, IMG_W, 3), np.float32)
    for core in range(NCORES):
        v, q = divmod(core, 4)
        full[v, 0, ROWS_PER_CORE * q:ROWS_PER_CORE * (q + 1)] = outs[core]
    return full, res


def kernel(grids, coords, rgb):
    full, _ = _run(np.asarray(grids), np.asarray(coords), np.asarray(rgb))
    return full
